# revision 43
# baseline (speedup 1.0000x reference)
"""Causal single-head attention (B=8, T=2048, E=1024, H=64) on 8 trn2 cores.

Sharding: data-parallel over batch; core b computes batch b end-to-end.

Device algorithm (per core) — v2, software-pipelined proj/attn interleave:
  xT [E,T] arrives pre-transposed from host (layout marshalling) so the
  E-contraction of the QKV projections has E on SBUF partitions.
  - Short HAM warm-up bridge (wz memset on gpsimd so it can start ~6us),
    then the projection matmuls themselves sustain PE activity; the HAM
    un-throttle latches ~3.4us after dense activity begins.
  - Projections, two full passes with packed 128-row stationaries:
      pass1 [Wq|Wk] -> qk1[128,T]: rows 0:64 q^T, 64:128 k^T
      pass2 [Wv|Wq] -> vq2[128,T]: rows 0:64 v^T, 64:128 q^T
    kk2[64,T] = copy of k^T to partitions 0:64 via SBUF->SBUF DMA (SWDGE).
    q^T and k^T on BOTH partition halves enable row-packed score matmuls.
    v is PE-transposed per 128-tile into natural [tk,64]; a ones column is
    appended -> vA [tk, 65].
  - Scores TRANSPOSED: ST[tk,tq] = k^T(tile).T @ q^T. Key-tile PAIRS are
    row-packed on disjoint 64-row groups of the PE array (concurrent MMs
    into different PSUM banks). |S/32| <= ~0.6 -> exp needs no row-max.
  - One exp per 2-tile group on ACT. Causality: groups above the diagonal
    skipped; diagonal-straddling groups trimmed at group granularity; the
    ragged 128-wide diagonal block of each tile is masked by a single
    [128,128] triangle dmT (c >= p) — only 128 columns multiplied.
  - oT[65,tq] accumulates over k-tiles: lhsT = [v | ones][128,65], rhs = P.
    Row 64 is the softmax denominator.
  - Epilogue: oT is PE-transposed in stride-2 column pairs so each output
    partition holds two ADJACENT tokens -> contiguous 512B DRAM lines,
    issued per 256-token chunk as soon as it is normalized.
  - EMISSION SCHEDULE: per superblock q, proj(q+1)'s qk pass runs as a
    DENSE PE block before attn(q) (covering the 2-4us latency of the
    kk2 SBUF->SBUF copy, whose descriptors queue behind bulk x DMA
    traffic), then attn(q)'s groups interleave with proj(q+1)'s vq pass
    so the PE has proj work queued during every exp wait and the ACT
    engine's exp stream starts ~2us earlier than a phase-ordered
    schedule. Pattern per group g:
        scores(g+1) | a couple proj MMs | outs(g)
    attn(3) (no proj left) keeps the split-PSUM half-epilogue so the left
    half's normalize+DMA overlaps the diagonal-B group.
    (Measured dead ends: two-group-deep attn pipelining, staggering the
    first scores into the qk block, splitting the warm-up into bursts,
    and splitting xt desc-gen across gpsimd all LOST time — mid-kernel
    HAM re-throttles and a later exp stream outweighed the theory.)
  - The key_padding_mask is all-ones for this workload; kernel() checks
    and compiles the mask-free variant (a masked variant folds the mask
    into vA rows, killing both numerator and denominator contributions).
"""

import numpy as np

import concourse.bass as bass
import concourse.mybir as mybir
import concourse.tile as tile
from concourse import bacc
from concourse.bass_utils import run_bass_kernel_spmd
from concourse.masks import make_identity

B, T, E, H = 8, 2048, 1024, 64
NQ = 512              # query superblock (columns of ST / oT)
N_QSB = T // NQ       # 4
N_KT = T // 128       # 16 key tiles
N_ET = E // 128       # 8 contraction tiles
NG = 2                # key tiles per exp group (2 fp32 PSUM banks)
SCALE = float(E) ** -0.5
N_WARM = 76           # contiguous HAM warm-up: a DENSE ~4.3us MM stream
                      # (gpsimd wz memset lets it start ~6us) so the PE
                      # un-throttle latches BEFORE the first proj MM; the
                      # bursts-between-DMA-paced-MMs variant left gaps
                      # that kept the busy-window from ever latching

MM_DT = mybir.dt.float16

_CACHE = {}


def _build(masked=False):
    f32 = mybir.dt.float32
    nc = bacc.Bacc("TRN2", target_bir_lowering=False)
    mmdt = MM_DT
    xT_d = nc.dram_tensor("xT", [E, T], mmdt, kind="ExternalInput")
    # weights host-prearranged partition-major [p, et, m]: contiguous
    # per-partition DRAM lines (big DMA descriptors)
    wqk_d = nc.dram_tensor("wqk", [128, N_ET, 128], mmdt, kind="ExternalInput")
    wvq_d = nc.dram_tensor("wvq", [128, N_ET, 128], mmdt, kind="ExternalInput")
    if masked:
        km_d = nc.dram_tensor("kmask", [T], f32, kind="ExternalInput")
    out_d = nc.dram_tensor("out", [T, H], f32, kind="ExternalOutput")

    with tile.TileContext(nc) as tc:
        with (
            tc.tile_pool(name="consts", bufs=1) as consts,
            tc.tile_pool(name="xt", bufs=3) as xt_pool,
            tc.tile_pool(name="big", bufs=1) as big,
            tc.tile_pool(name="pt", bufs=5) as pt_pool,
            tc.tile_pool(name="otsb", bufs=2) as otsb_pool,
            tc.tile_pool(name="osb", bufs=4) as osb_pool,
            tc.tile_pool(name="small", bufs=4) as small_pool,
            tc.tile_pool(name="warm", bufs=1) as warm_pool,
            tc.tile_pool(name="ps_proj", bufs=2, space="PSUM") as ps_proj,
            tc.tile_pool(name="ps_st", bufs=2, space="PSUM") as ps_st,
            tc.tile_pool(name="ps_ot", bufs=2, space="PSUM") as ps_ot,
        ):
            # ---- HAM warm-up bridge: gpsimd memset so PE can start ASAP
            wz = warm_pool.tile([128, 64], mmdt)
            nc.gpsimd.memset(wz, 0.0)
            wps = ps_proj.tile([64, 64], f32, tag="proj")

            def warm_mm():
                nc.tensor.matmul(
                    wps, lhsT=wz, rhs=wz[:, 0:64], start=True, stop=True
                )

            for _ in range(N_WARM):
                warm_mm()

            # ---- input DMAs. Each dma_start costs ~0.6us of descriptor
            # generation on its issuing HWDGE sequencer. sync is the main
            # x pump; scalar (idle until the first exp) takes the weight
            # chunks plus the xt0 tail so xt0 is fully generated ~2.5us
            # sooner; gpsimd (SWDGE, slow start) only carries wvq which
            # is not needed until p0 pass2.
            xt0 = xt_pool.tile([128, N_ET, NQ], mmdt, tag="xt")
            xt1 = xt_pool.tile([128, N_ET, NQ], mmdt, tag="xt")

            def xt_dma(eng, t, tb, et):
                eng.dma_start(
                    out=t[:, et, :],
                    in_=xT_d[et * 128 : (et + 1) * 128, bass.ts(tb, NQ)],
                )

            wqk_sb = consts.tile([128, N_ET, 128], mmdt)
            wvq_sb = consts.tile([128, N_ET, 128], mmdt)
            for et in range(6):
                xt_dma(nc.sync, xt0, 0, et)
            nc.scalar.dma_start(out=wqk_sb[:, 0:1, :], in_=wqk_d[:, 0:1, :])
            nc.scalar.dma_start(out=wqk_sb[:, 1:N_ET, :], in_=wqk_d[:, 1:N_ET, :])
            xt_dma(nc.scalar, xt0, 0, 6)
            xt_dma(nc.scalar, xt0, 0, 7)
            nc.gpsimd.dma_start(out=wvq_sb[:, 0:1, :], in_=wvq_d[:, 0:1, :])
            nc.gpsimd.dma_start(out=wvq_sb[:, 1:N_ET, :], in_=wvq_d[:, 1:N_ET, :])
            for et in range(N_ET):
                xt_dma(nc.sync, xt1, 1, et)
            if masked:
                km_sb = consts.tile([128, N_KT], f32)
                nc.scalar.dma_start(
                    out=km_sb, in_=km_d[:].rearrange("(kt p) -> p kt", p=128)
                )

            # ---- constants (gpsimd; ready well before first use) ----
            ident = consts.tile([128, 128], mmdt)
            make_identity(nc, ident)
            # diagonal-block triangle: dmT[p, c] = 1 iff c >= p
            dmT = consts.tile([128, 128], mmdt)
            nc.gpsimd.memset(dmT, 0.0)
            nc.gpsimd.affine_select(
                out=dmT,
                in_=dmT,
                compare_op=mybir.AluOpType.is_gt,
                fill=1.0,
                base=0,
                pattern=[[-1, 128]],
                channel_multiplier=1,
            )

            qk1 = big.tile([128, T], mmdt)  # rows 0:64 q^T, 64:128 k^T
            vq2 = big.tile([128, T], mmdt)  # rows 0:64 v^T, 64:128 q^T
            kk2 = big.tile([64, T], mmdt)   # k^T on partitions 0:64
            vA = big.tile([128, N_KT, H + 1], mmdt)  # v natural + ones col
            nc.vector.memset(vA[:, :, H : H + 1], 1.0)

            # ---- projection unit factories (emitted piecewise so attn
            # groups can interleave between chunks) ----
            def make_proj_qk(tb, xt):
                tsl = bass.ts(tb, NQ)
                state = {}

                def mm(et):
                    if et == 0:
                        state["ps"] = ps_proj.tile(
                            [128, NQ], f32, tag="proj", name="qk_ps"
                        )
                    nc.tensor.matmul(
                        state["ps"],
                        lhsT=wqk_sb[:, et, :],
                        rhs=xt[:, et, :],
                        start=(et == 0),
                        stop=(et == N_ET - 1),
                    )

                def fin():
                    nc.vector.tensor_copy(qk1[:, tsl], state["ps"])
                    # k^T also on partitions 0:64 (cross-partition: DMA on
                    # the gpsimd/SWDGE queue; sync queue carries x traffic)
                    nc.gpsimd.dma_start(out=kk2[:, tsl], in_=qk1[64:128, tsl])

                return [lambda et=et: mm(et) for et in range(N_ET)] + [fin]

            def make_proj_vq(tb, xt):
                tsl = bass.ts(tb, NQ)
                state = {}

                def mm(et):
                    if et == 0:
                        state["ps"] = ps_proj.tile(
                            [128, NQ], f32, tag="proj", name="vq_ps"
                        )
                    nc.tensor.matmul(
                        state["ps"],
                        lhsT=wvq_sb[:, et, :],
                        rhs=xt[:, et, :],
                        start=(et == 0),
                        stop=(et == N_ET - 1),
                    )

                def fin():
                    nc.vector.tensor_copy(vq2[:, tsl], state["ps"])

                def vtr(kt):
                    # v natural tile (+ mask folded into [v | ones] rows)
                    vps = ps_proj.tile([128, H], mmdt, tag="proj")
                    nc.tensor.transpose(
                        vps,
                        vq2[0:64, kt * 128 : (kt + 1) * 128],
                        ident[0:64, 0:64],
                    )
                    nc.vector.tensor_copy(vA[:, kt, 0:H], vps)
                    if masked:
                        nc.vector.tensor_scalar_mul(
                            vA[:, kt, :], vA[:, kt, :], km_sb[:, kt : kt + 1]
                        )

                return (
                    [lambda et=et: mm(et) for et in range(N_ET)]
                    + [fin]
                    + [lambda kt=kt: vtr(kt) for kt in range(4 * tb, 4 * tb + 4)]
                )

            def epi_half(qsb, s, ot_half):
                # epilogue for a 256-token half: transpose in stride-2
                # column pairs so each partition gets two adjacent
                # tokens (contiguous 512B DRAM lines)
                q0 = qsb * NQ
                otsb = otsb_pool.tile([H + 1, NQ // 2], mmdt, tag="otsb")
                nc.vector.tensor_copy(otsb, ot_half)
                otv = otsb.rearrange("p (t two) -> p two t", two=2, t=128)
                osb = osb_pool.tile([128, 2, H], f32, tag="osb")
                # The very LAST half (end of the serial kernel tail)
                # stores each 128-token piece as its own DMA on separate
                # engines, so piece 0's store overlaps piece 1's
                # normalize instead of waiting for it.
                last = qsb == N_QSB - 1 and s == 1
                outv = out_d[
                    q0 + 256 * s : q0 + 256 * (s + 1), :
                ].rearrange("(p two) h -> p two h", p=128)
                for par in range(2):
                    ott = ps_proj.tile([128, H + 1], mmdt, tag="proj")
                    nc.tensor.transpose(
                        ott, otv[:, par, :], ident[0 : H + 1, 0 : H + 1]
                    )
                    rec = small_pool.tile([128, 1], f32, tag="rec")
                    nc.vector.reciprocal(rec, ott[:, H : H + 1])
                    nc.vector.tensor_scalar_mul(
                        osb[:, par, :], ott[:, 0:H], rec
                    )
                    if last:
                        eng = nc.gpsimd if par == 0 else nc.sync
                        eng.dma_start(
                            out=outv[:, par, :], in_=osb[:, par, :]
                        )
                if not last:
                    out_eng = nc.sync if qsb == N_QSB - 1 else nc.gpsimd
                    out_eng.dma_start(out=outv, in_=osb)

            # ---- attention unit factory: scores(g) / outs(g) / epi ----
            def make_attn(qsb):
                q0 = qsb * NQ
                kt_last = 4 * qsb + 3
                # For the LAST superblock, oT accumulates in two
                # half-width tiles (separate PSUM banks) so the left
                # half's epilogue + DMA overlap the diagonal-B group.
                split = qsb == N_QSB - 1
                st = {"pts": {}}

                def alloc_ot():
                    # lazy: bind PSUM slots at first-use emission order
                    if split:
                        if "otL" not in st:
                            st["otL"] = ps_ot.tile(
                                [H + 1, NQ // 2], f32, tag="ot", name="otL"
                            )
                            st["otR"] = ps_ot.tile(
                                [H + 1, NQ // 2], f32, tag="ot", name="otR"
                            )
                    elif "ot" not in st:
                        st["ot"] = ps_ot.tile(
                            [H + 1, NQ], f32, tag="ot", name="ot_ps"
                        )

                def out_mm(kt, c0, rhs_pt):
                    alloc_ot()
                    start = kt == 0
                    if not split:
                        nc.tensor.matmul(
                            st["ot"][:, c0:],
                            lhsT=vA[:, kt, :],
                            rhs=rhs_pt,
                            start=start,
                            stop=(kt == kt_last),
                        )
                        return
                    if c0 < 256:
                        nc.tensor.matmul(
                            st["otL"][:, c0:],
                            lhsT=vA[:, kt, :],
                            rhs=rhs_pt[:, 0 : 256 - c0],
                            start=start,
                            stop=(kt == 4 * qsb + 1),
                        )
                        nc.tensor.matmul(
                            st["otR"],
                            lhsT=vA[:, kt, :],
                            rhs=rhs_pt[:, 256 - c0 :],
                            start=start,
                            stop=(kt == kt_last),
                        )
                    else:
                        nc.tensor.matmul(
                            st["otR"][:, c0 - 256 :],
                            lhsT=vA[:, kt, :],
                            rhs=rhs_pt,
                            start=start,
                            stop=(kt == kt_last),
                        )

                def scores(g):
                    # Row-packed pair: even tile on PE rows 0:63 (lhsT =
                    # kk2 tile, rhs = qk1 low = q), odd tile on rows
                    # 64:127 (lhsT = qk1 high tile = k, rhs = vq2 high =
                    # q) -> the two MMs run CONCURRENTLY on disjoint
                    # 64-row groups, into different PSUM banks.
                    dj = g - 2 * qsb
                    cg = 256 * dj if dj >= 0 else 0
                    stg = ps_st.tile([128, NG, NQ], f32, tag="st")
                    pt = pt_pool.tile([128, NG, NQ], mmdt, tag="pt")
                    kt = NG * g
                    nc.tensor.matmul(
                        stg[:, 0, cg:],
                        lhsT=kk2[:, kt * 128 : (kt + 1) * 128],
                        rhs=qk1[0:64, q0 + cg : q0 + NQ],
                        start=True,
                        stop=True,
                    )
                    nc.tensor.matmul(
                        stg[:, 1, cg:],
                        lhsT=qk1[64:128, (kt + 1) * 128 : (kt + 2) * 128],
                        rhs=vq2[64:128, q0 + cg : q0 + NQ],
                        start=True,
                        stop=True,
                    )
                    nc.scalar.activation(
                        pt[:, :, cg:],
                        stg[:, :, cg:],
                        mybir.ActivationFunctionType.Exp,
                        scale=SCALE,
                    )
                    st["pts"][g] = pt

                def outs(g):
                    dj = g - 2 * qsb
                    pt = st["pts"].pop(g)
                    for j in range(NG):
                        kt = NG * g + j
                        c0 = 128 * (2 * dj + j) if dj >= 0 else 0
                        if dj >= 0:
                            # ragged diagonal block: only the 128-wide
                            # block [c0, c0+128) needs the triangle mask
                            # (columns beyond it are fully valid)
                            nc.vector.tensor_mul(
                                pt[:, j, c0 : c0 + 128],
                                pt[:, j, c0 : c0 + 128],
                                dmT,
                            )
                        out_mm(kt, c0, pt[:, j, c0:])

                def epi(s):
                    if split:
                        epi_half(qsb, s, st["otL"] if s == 0 else st["otR"])
                    else:
                        epi_half(
                            qsb,
                            s,
                            st["ot"][:, 0 : NQ // 2]
                            if s == 0
                            else st["ot"][:, NQ // 2 :],
                        )

                return scores, outs, epi

            def xt_fetch(tb):
                t = xt_pool.tile([128, N_ET, NQ], mmdt, tag="xt")
                for et in range(N_ET):
                    xt_dma(nc.sync, t, tb, et)
                return t

            # ---- pipelined main program ----
            # attn(q) units interleaved with proj(q+1) units: after each
            # scores(g), a slice of proj MMs fills the PE while ACT runs
            # exp(g); outs(g) follows.
            def interleave(attn_units, filler_units):
                """Emit attn_units in order, spreading filler_units
                evenly into the slots between them."""
                n_a = len(attn_units)
                n_f = len(filler_units)
                fi = 0
                for i, u in enumerate(attn_units):
                    u()
                    want = (i + 1) * n_f // n_a
                    while fi < want:
                        filler_units[fi]()
                        fi += 1
                while fi < n_f:
                    filler_units[fi]()
                    fi += 1

            # p0: pass1 is paced by the xt0 DMA stream (runs warm: the
            # warm-up block above latched the un-throttle already)
            for u in make_proj_qk(0, xt0):
                u()
            for u in make_proj_vq(0, xt0):
                u()

            s0, o0, e0 = make_attn(0)
            s1, o1, e1 = make_attn(1)
            s2, o2, e2 = make_attn(2)
            s3, o3, e3 = make_attn(3)

            # attn(q) phases: p(q+1)'s qk pass runs as a DENSE block
            # before attn(q) — it needs no attn inputs and covers the
            # ~2-4us latency of the kk2 SBUF->SBUF copy (whose DMA
            # descriptors queue behind the bulk x traffic). Only the vq
            # pass interleaves with attn(q)'s exp-wait windows.
            for u in make_proj_qk(1, xt1):
                u()
            a0 = [
                lambda: s0(0),
                lambda: s0(1),
                lambda: o0(0),
                lambda: o0(1),
                lambda: e0(0),
                lambda: e0(1),
            ]
            xt2 = xt_fetch(2)
            interleave(a0, make_proj_vq(1, xt1))

            for u in make_proj_qk(2, xt2):
                u()
            a1 = [
                lambda: s1(0),
                lambda: s1(1),
                lambda: o1(0),
                lambda: s1(2),
                lambda: o1(1),
                lambda: s1(3),
                lambda: o1(2),
                lambda: o1(3),
                lambda: e1(0),
                lambda: e1(1),
            ]
            xt3 = xt_fetch(3)
            interleave(a1, make_proj_vq(2, xt2))
            # (Weaving s2(0..1) into this interleave after p2vq-fin —
            # the v10 pattern one phase earlier — measured neutral:
            # the ~5us ACT idle it removes is not on the critical path;
            # the kernel end is PE-bound through s3(7).)

            for u in make_proj_qk(3, xt3):
                u()
            # attn(2) interleaves with p3vq; attn(3)'s first score
            # groups are woven INTO that interleave right after p3vq's
            # finish (their only dependency is the vq2/q3-high copy +
            # kk2(3), both covered by then): the measured ~3us of ACT
            # idle before exp(3,0) disappears and the ACT-bound a3 tail
            # (8 exps vs ~6us of PE work) starts draining during a2.
            # a2's last out-groups + epilogue give the PE filler under
            # a3's first exps. (Weaving s(q,0..1) into the p(q+1)qk
            # blocks the same way was measured WORSE — scores at the
            # block head stall the dense qk block on kk2/PSUM sems.)
            a2_units = [
                lambda: s2(0),
                lambda: s2(1),
                lambda: o2(0),
                lambda: s2(2),
                lambda: o2(1),
                lambda: s2(3),
                lambda: o2(2),
                lambda: s2(4),
                lambda: o2(3),
                lambda: s3(0),
                lambda: s2(5),
                lambda: s3(1),
            ]
            interleave(a2_units, make_proj_vq(3, xt3))
            o2(4)
            s3(2)
            o2(5)
            e2(0)
            e2(1)
            o3(0)
            s3(3)
            o3(1)
            s3(4)
            o3(2)
            s3(5)
            o3(3)
            s3(6)
            o3(4)
            s3(7)
            o3(5)
            o3(6)
            e3(0)
            o3(7)
            e3(1)

    nc.finalize()
    return nc


def get_nc(masked=False):
    key = ("nc", masked)
    if key not in _CACHE:
        _CACHE[key] = _build(masked)
    return _CACHE[key]


def make_in_maps(x, Wq, Wk, Wv, key_padding_mask, masked):
    np_dt = np.float16 if MM_DT == mybir.dt.float16 else np.float32

    def prearrange(w):  # [E, 128] -> [128, N_ET, 128] partition-major
        return np.ascontiguousarray(
            w.astype(np_dt).reshape(N_ET, 128, 128).transpose(1, 0, 2)
        )

    x = np.asarray(x, dtype=np.float32)
    Wq, Wk, Wv = (np.asarray(w) for w in (Wq, Wk, Wv))
    wqk = prearrange(np.concatenate([Wq, Wk], axis=1))
    wvq = prearrange(np.concatenate([Wv, Wq], axis=1))
    xT = np.ascontiguousarray(x.transpose(0, 2, 1).astype(np_dt))  # [B, E, T]
    maps = []
    for b in range(B):
        m = {"xT": xT[b], "wqk": wqk, "wvq": wvq}
        if masked:
            m["kmask"] = np.asarray(key_padding_mask)[b].astype(np.float32)
        maps.append(m)
    return maps


def kernel(x, Wq, Wk, Wv, key_padding_mask, _trace=False, _trace_cores=None):
    masked = not bool(np.all(np.asarray(key_padding_mask)))
    nc = get_nc(masked)
    in_maps = make_in_maps(x, Wq, Wk, Wv, key_padding_mask, masked)
    res = run_bass_kernel_spmd(
        nc,
        in_maps,
        core_ids=list(range(B)),
        trace=_trace,
        trace_cores=_trace_cores,
    )
    _CACHE["last_results"] = res
    return np.stack([res.results[b]["out"] for b in range(B)], axis=0)
